# revision 28
# baseline (speedup 1.0000x reference)
"""Trainium2 Bass kernel for nn_DiagonalLinear.

Reference op: y = x @ (W * eye * (|W*eye| > 0.001)).T  — i.e. an
elementwise column scale y[b, o] = x[b, o] * d[o] with
d[o] = W[o, o] if |W[o, o]| > 0.001 else 0.

Sharding: data-parallel over batch; each of the 8 cores handles a
contiguous (1024, 4096) slice of x plus the replicated 4096-entry
diagonal of W. The op is pure HBM bandwidth, so the kernel moves x/y
in reduced precision (well inside the 2e-2 rel-err budget):

  mode "fp16": x staged fp16, y returned fp16      -> 16 MiB/core
  mode "int8": x staged as int8 codes with f32 per-column scales
               (folded into d on device), y fp16   -> 12 MiB/core

versus 32 MiB/core for the all-f32 baseline. The threshold mask and
the scale folding are applied on-device; each x tile is a DMA-in /
multiply / DMA-out pipeline. In int8 mode the multiply runs at 1
elem/lane/cycle on DVE, so a slice of row blocks is offloaded to
gpsimd to keep the multiply off the critical path.
"""

import numpy as np

import concourse.bacc as bacc
import concourse.mybir as mybir
from concourse.bass_utils import run_bass_kernel_spmd
from concourse.tile import TileContext

N = 4096          # feature dim
B = 8192          # batch
NCORES = 8
BS = B // NCORES  # 1024 rows per core
P = 128           # SBUF partitions
ROW_BLOCKS = BS // P          # 8 blocks of 128 rows per core
THRESHOLD = 0.001
F32 = mybir.dt.float32
F16 = mybir.dt.float16
I8 = mybir.dt.int8

MODE = "int8"     # "fp16" | "int8" | "int8o"(int8-out: walrus crash, unsupported)
FUSE = 1          # row blocks fused per SBUF tile / DMA
BUFS = 12
GPS_EVERY = 4     # int8 mode: every GPS_EVERY-th row block multiplies on gpsimd
LOAD_ENG = "sync"     # loads on the SP HWDGE ring
STORE_ENG = "scalar"  # stores on the ACT HWDGE ring (unidirectional rings
                      # unlock duplex DMA; mixing directions on one ring
                      # serializes on HBM turnaround/completion receipts)

LAST_RESULTS = None


def in_bytes(mode=MODE):
    return BS * N * (2 if mode == "fp16" else 1)


def out_bytes(mode=MODE):
    return BS * N * (1 if mode == "int8o" else 2)


def build_nc(repeat=1, fuse=FUSE, bufs=BUFS, mode=MODE, gps_every=GPS_EVERY,
             load_eng=LOAD_ENG, store_eng=STORE_ENG, lsplit=1, ssplit=1,
             body="normal"):
    ntiles = ROW_BLOCKS // fuse
    nc = bacc.Bacc()
    int8_in = mode in ("int8", "int8o")
    ydt = I8 if mode == "int8o" else F16

    def eng_for(which, t):
        if which == "alt":       # even tiles sync, odd scalar
            return nc.sync if t % 2 == 0 else nc.scalar
        if which == "alt2":      # even tiles scalar, odd sync
            return nc.scalar if t % 2 == 0 else nc.sync
        return getattr(nc, which)
    xdt = F16 if mode == "fp16" else I8
    x_in = nc.declare_dram_parameter("x", [BS, N], xdt, isOutput=False)
    d_in = nc.declare_dram_parameter("d", [1, N], F32, isOutput=False)
    s_in = (nc.declare_dram_parameter("s", [1, N], F32, isOutput=False)
            if int8_in else None)
    y_out = nc.declare_dram_parameter("y", [BS, N], ydt, isOutput=True)

    # [BS, N] viewed as [P, ROW_BLOCKS, N]: row r = n*P + p
    x_v = x_in[:].rearrange("(n p) d -> p n d", p=P)
    y_v = y_out[:].rearrange("(n p) d -> p n d", p=P)

    with TileContext(nc) as tc:
        with (
            tc.tile_pool(name="const", bufs=1) as cpool,
            tc.tile_pool(name="in", bufs=bufs) as inpool,
            tc.tile_pool(name="out", bufs=bufs) as outpool,
            tc.tile_pool(name="ps", bufs=8, space="PSUM") as pspool,
        ):
            # Broadcast the 16 KB diagonal row (and in int8 mode the
            # dequant scales) to all 128 partitions with a PE matmul by a
            # ones matrix against a one-hot-row rhs (bit-exact: every
            # product is 1.0*v or 1.0*0.0). Then apply the |d| > threshold
            # mask, fold in the scales, and round to the multiply dtype.
            ones = cpool.tile([P, P], F32)
            nc.vector.memset(ones[:], 1.0)
            CH = 512  # PSUM bank free-dim capacity (f32)

            def bcast_row(dram_row, out):
                # rhs/scratch tiles share one slot (same tag, bufs=1 pool):
                # broadcasts are sequential, so rotation just serializes them.
                rhs = cpool.tile([P, N], F32, name="rhs", tag="rhs")
                nc.vector.memset(rhs[:], 0.0)
                nc.sync.dma_start(out=rhs[0:1, :], in_=dram_row)
                for c in range(N // CH):
                    sl = slice(c * CH, (c + 1) * CH)
                    acc = pspool.tile([P, CH], F32, name="acc", tag="acc")
                    nc.tensor.matmul(acc[:], ones[:], rhs[:, sl],
                                     start=True, stop=True)
                    nc.vector.tensor_copy(out[:, sl], acc[:])
                return out

            dbc = bcast_row(d_in[:], cpool.tile([P, N], F32, name="bc_d"))
            tmp = cpool.tile([P, N], F32, name="scr", tag="scr")
            nc.vector.tensor_scalar(
                tmp[:], dbc[:], -1.0, None, mybir.AluOpType.mult
            )
            nc.vector.tensor_tensor(
                tmp[:], dbc[:], tmp[:], mybir.AluOpType.max
            )
            nc.vector.scalar_tensor_tensor(
                dbc[:], tmp[:], THRESHOLD, dbc[:],
                mybir.AluOpType.is_gt, mybir.AluOpType.mult,
            )
            if int8_in:
                sbc = bcast_row(
                    s_in[:], cpool.tile([P, N], F32, name="scr", tag="scr")
                )
                nc.vector.tensor_tensor(
                    dbc[:], dbc[:], sbc[:], mybir.AluOpType.mult
                )
                dmul = dbc  # f32; int8 path runs 1x on DVE regardless
            else:
                # fp16 multiplier: with both TT operands 16-bit the DVE
                # runs 2x_1P (2 elem/lane/cycle)
                dmul = cpool.tile([P, N], F16)
                nc.vector.tensor_copy(dmul[:], dbc[:])

            if body == "mult":
                # Engine-rate microbenchmark: per repeat, ROW_BLOCKS
                # multiplies with no DMA. Separate out tiles per engine so
                # WAW serializes only within an engine. Bench-only mode
                # (y is never written).
                mi = cpool.tile([P, N], xdt, name="mi")
                nc.sync.dma_start(out=mi[:], in_=x_v[:, 0, :])
                mo_v = cpool.tile([P, N], F16, name="mo_v")
                mo_g = cpool.tile([P, N], F16, name="mo_g")
                for _ in range(repeat):
                    for blk in range(ROW_BLOCKS):
                        if int8_in and blk % gps_every == gps_every - 1:
                            nc.gpsimd.tensor_tensor(
                                mo_g[:], mi[:], dmul[:], mybir.AluOpType.mult)
                        else:
                            nc.vector.tensor_tensor(
                                mo_v[:], mi[:], dmul[:], mybir.AluOpType.mult)
            elif body == "dma":
                # DMA-rate microbenchmark: loads + stores, no compute.
                # Stores push whatever the out tiles hold (bench-only).
                for _ in range(repeat):
                    for t in range(ntiles):
                        ts = slice(t * fuse, (t + 1) * fuse)
                        tl = inpool.tile([P, fuse, N], xdt, name="tl")
                        eng_for(load_eng, t).dma_start(
                            out=tl[:], in_=x_v[:, ts, :])
                        ot = outpool.tile([P, fuse, N], ydt, name="ot")
                        nc.vector.memset(ot[:], 0.0)
                        eng_for(store_eng, t).dma_start(
                            out=y_v[:, ts, :], in_=ot[:])
            else:
                for _ in range(repeat):
                    for t in range(ntiles):
                        ts = slice(t * fuse, (t + 1) * fuse)
                        tl = inpool.tile([P, fuse, N], xdt, name="tl")
                        for c in range(lsplit):
                            cs = slice(c * N // lsplit, (c + 1) * N // lsplit)
                            eng_for(load_eng, t).dma_start(
                                out=tl[:, :, cs], in_=x_v[:, ts, cs]
                            )
                        ot = tl if mode == "fp16" else outpool.tile(
                            [P, fuse, N], ydt, name="ot"
                        )
                        for j in range(fuse):
                            blk = t * fuse + j
                            eng = nc.vector
                            if (int8_in
                                    and blk % gps_every == gps_every - 1):
                                eng = nc.gpsimd
                            eng.tensor_tensor(
                                ot[:, j, :], tl[:, j, :], dmul[:],
                                mybir.AluOpType.mult,
                            )
                        for c in range(ssplit):
                            cs = slice(c * N // ssplit, (c + 1) * N // ssplit)
                            eng_for(store_eng, t).dma_start(
                                out=y_v[:, ts, cs], in_=ot[:, :, cs]
                            )
    nc.finalize()
    return nc


def _prep(x, W, mode=MODE):
    """Returns (in_maps, decode) — decode is the per-column host-side
    dequant vector for the output (None unless mode == "int8o")."""
    x = np.ascontiguousarray(np.asarray(x, dtype=np.float32))
    d = np.ascontiguousarray(np.diagonal(W)).astype(np.float32).reshape(1, N)
    decode = None
    if mode == "fp16":
        xq = x.astype(np.float16)
        extra = {}
    else:
        absmax = np.abs(x).max(axis=0)
        s = (absmax / 127.0).astype(np.float32)
        s[s == 0] = 1.0
        xq = np.rint(x * (1.0 / s)).astype(np.int8)
        if mode == "int8o":
            # MXINT8-style output: int8 mantissas + per-column shared pow2
            # exponent. The host picks e[o] (quantization metadata) so
            # |code| <= 127; the device applies the full masked-d mantissa
            # via s2 = s * 2^-e and the host decode is an exact 2^e shift.
            dm = d.ravel() * (np.abs(d.ravel()) > THRESHOLD)
            M = 127.0 * s * np.abs(dm)
            e = np.where(M > 0, np.ceil(np.log2(np.maximum(M, 1e-30)
                                                / 127.49)), 0.0)
            p2 = np.exp2(e).astype(np.float32)
            s = (s / p2).astype(np.float32)
            decode = p2
        extra = {"s": s.reshape(1, N).astype(np.float32)}
    xs = xq.reshape(NCORES, BS, N)
    return [{"x": xs[i], "d": d, **extra} for i in range(NCORES)], decode


def make_in_maps(x, W, mode=MODE):
    return _prep(x, W, mode)[0]


def kernel(x: np.ndarray, W: np.ndarray) -> np.ndarray:
    global LAST_RESULTS
    in_maps, decode = _prep(x, W)
    nc = build_nc()
    res = run_bass_kernel_spmd(nc, in_maps, core_ids=list(range(NCORES)))
    LAST_RESULTS = res
    y = np.concatenate([r["y"] for r in res.results], axis=0)
    y = y.astype(np.float32)
    if decode is not None:
        y *= decode[None, :]
    return y


# revision 31
# speedup vs baseline: 1.0764x; 1.0764x over previous
"""Trainium2 Bass kernel for nn_DiagonalLinear.

Reference op: y = x @ (W * eye * (|W*eye| > 0.001)).T  — i.e. an
elementwise column scale y[b, o] = x[b, o] * d[o] with
d[o] = W[o, o] if |W[o, o]| > 0.001 else 0.

Sharding: data-parallel over batch; each of the 8 cores handles a
contiguous (1024, 4096) slice of x plus the replicated 4096-entry
diagonal of W. The op is pure HBM bandwidth, so the kernel moves x/y
in reduced precision (well inside the 2e-2 rel-err budget):

  mode "fp16": x staged fp16, y returned fp16      -> 16 MiB/core
  mode "int8": x staged as int8 codes with f32 per-column scales
               (folded into d on device), y fp16   -> 12 MiB/core

versus 32 MiB/core for the all-f32 baseline. The threshold mask and
the scale folding are applied on-device; each x tile is a DMA-in /
multiply / DMA-out pipeline. In int8 mode the multiply runs at 1
elem/lane/cycle on DVE, so a slice of row blocks is offloaded to
gpsimd to keep the multiply off the critical path.
"""

import numpy as np

import concourse.bacc as bacc
import concourse.mybir as mybir
from concourse.bass_utils import run_bass_kernel_spmd
from concourse.tile import TileContext

N = 4096          # feature dim
B = 8192          # batch
NCORES = 8
BS = B // NCORES  # 1024 rows per core
P = 128           # SBUF partitions
ROW_BLOCKS = BS // P          # 8 blocks of 128 rows per core
THRESHOLD = 0.001
F32 = mybir.dt.float32
F16 = mybir.dt.float16
I8 = mybir.dt.int8

MODE = "int8"     # "fp16" | "int8" | "int8o"(int8-out: walrus crash, unsupported)
FUSE = 1          # row blocks fused per SBUF tile / DMA
BUFS = 12
GPS_EVERY = 4     # int8 mode: every GPS_EVERY-th row block multiplies on gpsimd
LOAD_ENG = "sync"     # loads on the SP HWDGE ring
STORE_ENG = "scalar"  # stores on the ACT HWDGE ring (unidirectional rings
                      # unlock duplex DMA; mixing directions on one ring
                      # serializes on HBM turnaround/completion receipts)

LAST_RESULTS = None


def in_bytes(mode=MODE):
    return BS * N * (2 if mode == "fp16" else 1)


def out_bytes(mode=MODE):
    return BS * N * (1 if mode == "int8o" else 2)


def build_nc(repeat=1, fuse=FUSE, bufs=BUFS, mode=MODE, gps_every=GPS_EVERY,
             load_eng=LOAD_ENG, store_eng=STORE_ENG, lsplit=1, ssplit=1,
             body="normal", dmul16=1):
    ntiles = ROW_BLOCKS // fuse
    nc = bacc.Bacc()
    int8_in = mode in ("int8", "int8o")
    ydt = I8 if mode == "int8o" else F16

    def eng_for(which, t):
        if which == "alt":       # even tiles sync, odd scalar
            return nc.sync if t % 2 == 0 else nc.scalar
        if which == "alt2":      # even tiles scalar, odd sync
            return nc.scalar if t % 2 == 0 else nc.sync
        return getattr(nc, which)
    xdt = F16 if mode == "fp16" else I8
    x_in = nc.declare_dram_parameter("x", [BS, N], xdt, isOutput=False)
    d_in = nc.declare_dram_parameter("d", [1, N], F32, isOutput=False)
    s_in = (nc.declare_dram_parameter("s", [1, N], F32, isOutput=False)
            if int8_in else None)
    y_out = nc.declare_dram_parameter("y", [BS, N], ydt, isOutput=True)

    # [BS, N] viewed as [P, ROW_BLOCKS, N]: row r = n*P + p
    x_v = x_in[:].rearrange("(n p) d -> p n d", p=P)
    y_v = y_out[:].rearrange("(n p) d -> p n d", p=P)

    with TileContext(nc) as tc:
        with (
            tc.tile_pool(name="const", bufs=1) as cpool,
            tc.tile_pool(name="in", bufs=bufs) as inpool,
            tc.tile_pool(name="out", bufs=bufs) as outpool,
            tc.tile_pool(name="ps", bufs=8, space="PSUM") as pspool,
        ):
            # Broadcast the 16 KB diagonal row (and in int8 mode the
            # dequant scales) to all 128 partitions with a PE matmul by a
            # ones matrix against a one-hot-row rhs (bit-exact: every
            # product is 1.0*v or 1.0*0.0). Then apply the |d| > threshold
            # mask, fold in the scales, and round to the multiply dtype.
            ones = cpool.tile([P, P], F32)
            nc.vector.memset(ones[:], 1.0)
            CH = 512  # PSUM bank free-dim capacity (f32)

            def bcast_row(dram_row, out):
                # rhs/scratch tiles share one slot (same tag, bufs=1 pool):
                # broadcasts are sequential, so rotation just serializes them.
                rhs = cpool.tile([P, N], F32, name="rhs", tag="rhs")
                nc.vector.memset(rhs[:], 0.0)
                nc.sync.dma_start(out=rhs[0:1, :], in_=dram_row)
                for c in range(N // CH):
                    sl = slice(c * CH, (c + 1) * CH)
                    acc = pspool.tile([P, CH], F32, name="acc", tag="acc")
                    nc.tensor.matmul(acc[:], ones[:], rhs[:, sl],
                                     start=True, stop=True)
                    nc.vector.tensor_copy(out[:, sl], acc[:])
                return out

            dbc = bcast_row(d_in[:], cpool.tile([P, N], F32, name="bc_d"))
            tmp = cpool.tile([P, N], F32, name="scr", tag="scr")
            nc.vector.tensor_scalar(
                tmp[:], dbc[:], -1.0, None, mybir.AluOpType.mult
            )
            nc.vector.tensor_tensor(
                tmp[:], dbc[:], tmp[:], mybir.AluOpType.max
            )
            nc.vector.scalar_tensor_tensor(
                dbc[:], tmp[:], THRESHOLD, dbc[:],
                mybir.AluOpType.is_gt, mybir.AluOpType.mult,
            )
            if int8_in:
                sbc = bcast_row(
                    s_in[:], cpool.tile([P, N], F32, name="scr", tag="scr")
                )
                nc.vector.tensor_tensor(
                    dbc[:], dbc[:], sbc[:], mybir.AluOpType.mult
                )
                if dmul16:
                    # fp16 multiplier: probe whether the DVE picks a faster
                    # mode when in1/out are both 16-bit (in0 stays int8)
                    dmul = cpool.tile([P, N], F16, name="dmul16")
                    nc.vector.tensor_copy(dmul[:], dbc[:])
                else:
                    dmul = dbc  # f32 multiplier
            else:
                # fp16 multiplier: with both TT operands 16-bit the DVE
                # runs 2x_1P (2 elem/lane/cycle)
                dmul = cpool.tile([P, N], F16)
                nc.vector.tensor_copy(dmul[:], dbc[:])

            if body == "mult":
                # Engine-rate microbenchmark: per repeat, ROW_BLOCKS
                # multiplies with no DMA. Separate out tiles per engine so
                # WAW serializes only within an engine. Bench-only mode
                # (y is never written).
                mi = cpool.tile([P, N], xdt, name="mi")
                nc.sync.dma_start(out=mi[:], in_=x_v[:, 0, :])
                mo_v = cpool.tile([P, N], F16, name="mo_v")
                mo_g = cpool.tile([P, N], F16, name="mo_g")
                for _ in range(repeat):
                    for blk in range(ROW_BLOCKS):
                        if int8_in and blk % gps_every == gps_every - 1:
                            nc.gpsimd.tensor_tensor(
                                mo_g[:], mi[:], dmul[:], mybir.AluOpType.mult)
                        else:
                            nc.vector.tensor_tensor(
                                mo_v[:], mi[:], dmul[:], mybir.AluOpType.mult)
            elif body == "dma":
                # DMA-rate microbenchmark: loads + stores, no compute.
                # Stores push whatever the out tiles hold (bench-only).
                for _ in range(repeat):
                    for t in range(ntiles):
                        ts = slice(t * fuse, (t + 1) * fuse)
                        tl = inpool.tile([P, fuse, N], xdt, name="tl")
                        eng_for(load_eng, t).dma_start(
                            out=tl[:], in_=x_v[:, ts, :])
                        ot = outpool.tile([P, fuse, N], ydt, name="ot")
                        nc.vector.memset(ot[:], 0.0)
                        eng_for(store_eng, t).dma_start(
                            out=y_v[:, ts, :], in_=ot[:])
            else:
                for _ in range(repeat):
                    for t in range(ntiles):
                        ts = slice(t * fuse, (t + 1) * fuse)
                        tl = inpool.tile([P, fuse, N], xdt, name="tl")
                        for c in range(lsplit):
                            cs = slice(c * N // lsplit, (c + 1) * N // lsplit)
                            eng_for(load_eng, t).dma_start(
                                out=tl[:, :, cs], in_=x_v[:, ts, cs]
                            )
                        ot = tl if mode == "fp16" else outpool.tile(
                            [P, fuse, N], ydt, name="ot"
                        )
                        for j in range(fuse):
                            blk = t * fuse + j
                            eng = nc.vector
                            if (int8_in
                                    and blk % gps_every == gps_every - 1):
                                eng = nc.gpsimd
                            eng.tensor_tensor(
                                ot[:, j, :], tl[:, j, :], dmul[:],
                                mybir.AluOpType.mult,
                            )
                        for c in range(ssplit):
                            cs = slice(c * N // ssplit, (c + 1) * N // ssplit)
                            eng_for(store_eng, t).dma_start(
                                out=y_v[:, ts, cs], in_=ot[:, :, cs]
                            )
    nc.finalize()
    return nc


def _prep(x, W, mode=MODE):
    """Returns (in_maps, decode) — decode is the per-column host-side
    dequant vector for the output (None unless mode == "int8o")."""
    x = np.ascontiguousarray(np.asarray(x, dtype=np.float32))
    d = np.ascontiguousarray(np.diagonal(W)).astype(np.float32).reshape(1, N)
    decode = None
    if mode == "fp16":
        xq = x.astype(np.float16)
        extra = {}
    else:
        absmax = np.abs(x).max(axis=0)
        s = (absmax / 127.0).astype(np.float32)
        s[s == 0] = 1.0
        xq = np.rint(x * (1.0 / s)).astype(np.int8)
        if mode == "int8o":
            # MXINT8-style output: int8 mantissas + per-column shared pow2
            # exponent. The host picks e[o] (quantization metadata) so
            # |code| <= 127; the device applies the full masked-d mantissa
            # via s2 = s * 2^-e and the host decode is an exact 2^e shift.
            dm = d.ravel() * (np.abs(d.ravel()) > THRESHOLD)
            M = 127.0 * s * np.abs(dm)
            e = np.where(M > 0, np.ceil(np.log2(np.maximum(M, 1e-30)
                                                / 127.49)), 0.0)
            p2 = np.exp2(e).astype(np.float32)
            s = (s / p2).astype(np.float32)
            decode = p2
        extra = {"s": s.reshape(1, N).astype(np.float32)}
    xs = xq.reshape(NCORES, BS, N)
    return [{"x": xs[i], "d": d, **extra} for i in range(NCORES)], decode


def make_in_maps(x, W, mode=MODE):
    return _prep(x, W, mode)[0]


def kernel(x: np.ndarray, W: np.ndarray) -> np.ndarray:
    global LAST_RESULTS
    in_maps, decode = _prep(x, W)
    nc = build_nc()
    res = run_bass_kernel_spmd(nc, in_maps, core_ids=list(range(NCORES)))
    LAST_RESULTS = res
    y = np.concatenate([r["y"] for r in res.results], axis=0)
    y = y.astype(np.float32)
    if decode is not None:
        y *= decode[None, :]
    return y
